# revision 5
# baseline (speedup 1.0000x reference)
"""Batch-parallel attention kernel for 8 TRN2 NeuronCores.

Problem: q,k,v [32, 2048, 128] f32 -> out = softmax(q@k^T/sqrt(128)) @ v.

Sharding: batch dim across 8 cores (4 batches/core), no cross-core comm.

Per-core algorithm (per batch, N=2048, D=128); ScalarE exp is the pacing
engine (16.8M exps at 1 elem/cycle/lane @1.2GHz = 109.2us floor):
  - Scores S^T[k, q] per 512-col q-chunk on PE (K^T tile stationary,
    Q^T chunk streaming, f32 PSUM), exp'd by ScalarE in SIX activations
    per chunk (k-tile groups 3,3,3,3,2,2): TRN2 PSUM matmul output must
    be f32, so a 3-k-tile group (1536 f32 = 3 banks) double-buffered
    (6 banks) plus 2 MM2-accumulator banks exactly fills the 8-bank
    PSUM.  Fewer/larger ACTIVATEs amortize the ~144ns/instr overhead
    (997ns -> 1424ns per 1024 -> 1536 elems).
  - V_aug [k, t, D+1] with a ones column makes the softmax denominator
    fall out of MM2 (column 128) -- no cross-partition reduction.
  - MM2: out[q,129] accumulated over 16 k-tiles with P^T[k,q] tiles
    stationary (FWL keeps the ~59ns/matmul cadence), then VectorE
    reciprocal+scale.  MM2 chains sit in a rolling queue and are
    emitted into later chunks' PE slack (ScalarE paces at ~7.7us/chunk,
    PE needs ~7.2), so the batch-0 backlog drains smoothly.
  - Batch 0 ramp is k-major: q-chunks 0 and 1 are produced interleaved
    (qc1 staggered one k-group behind qc0) so each PE K^T-transpose
    feeds TWO exps; only kt0-5 / qt0-3 are transposed before the first
    exp, the rest drip in with a 2-group lead.  Q tiles 8-15 arrive via
    SWDGE f32->bf16 cast-DMA + xbar transpose-DMA (no PE/DVE work), as
    do all tensors of batches 1-3 (drip-issued with multi-chunk lead).
  - No max-subtraction: scores are ~N(0,1), exp is exact to ~2ulp on
    ScalarE and stays in range.

rel_l2 ~3e-3 vs f64 reference (bf16 operand rounding; f32 accumulation).
"""

import math
from collections import deque

import numpy as np

import concourse.bass as bass
import concourse.mybir as mybir
import concourse.tile as tile
from concourse import bacc
from concourse.bass import ts
from concourse.bass_utils import run_bass_kernel_spmd
from concourse.masks import make_identity

B, N, D = 32, 2048, 128
N_CORES = 8
B_LOC = B // N_CORES  # batches per core
NT = N // 128  # 16 k-tiles per batch
QCHUNK = 512
NQC = N // QCHUNK  # 4 q-chunks
SCALE = 1.0 / math.sqrt(D)
FP32 = mybir.dt.float32
BF16 = mybir.dt.bfloat16

# exp groups per chunk: (k-tile offset, n k-tiles). 3 k-tiles = 3 PSUM
# banks f32; double-buffered = 6 banks, + 2 MM2-acc banks = 8 total.
GROUPS = [(0, 3), (3, 3), (6, 3), (9, 3), (12, 2), (14, 2)]

_CACHE = {}


def build_nc():
    nc = bacc.Bacc(None, target_bir_lowering=False)
    q_d = nc.dram_tensor("q", [B_LOC, N, D], FP32, kind="ExternalInput")
    k_d = nc.dram_tensor("k", [B_LOC, N, D], FP32, kind="ExternalInput")
    v_d = nc.dram_tensor("v", [B_LOC, N, D], FP32, kind="ExternalInput")
    o_d = nc.dram_tensor("out", [B_LOC, N, D], FP32, kind="ExternalOutput")

    with tile.TileContext(nc) as tc:
        with (
            tc.tile_pool(name="const", bufs=1) as constp,
            tc.tile_pool(name="dram", bufs=2, space="DRAM") as dramp,
            tc.tile_pool(name="stg", bufs=6) as stg,
            tc.tile_pool(name="b16", bufs=6) as b16p,
            tc.tile_pool(name="big", bufs=2) as big,
            tc.tile_pool(name="pt", bufs=3) as ptp,
            tc.tile_pool(name="outp", bufs=3) as outp,
            tc.tile_pool(name="small", bufs=8) as smallp,
            tc.tile_pool(name="st", bufs=2, space="PSUM") as stp,
            tc.tile_pool(name="acc", bufs=2, space="PSUM") as accp,
        ):
            ident = constp.tile([128, 128], BF16)

            batch_tiles = {}

            # ---------------- batch 0 ramp helpers (PE transposes) -----
            b0 = {}

            def b0_load(key, src_d, t0, nt_):
                s = stg.tile([128, nt_, 128], FP32, tag="stg",
                             name=f"s_{key}_{t0}")
                nc.sync.dma_start(
                    s[:],
                    src_d[0, bass.ds(t0 * 128, nt_ * 128), :].rearrange(
                        "(t p) d -> p t d", p=128
                    ),
                )
                b0[("f32", key, t0)] = (s, nt_)

            def b0_cast(key, t0):
                s, nt_ = b0.pop(("f32", key, t0))
                c = b16p.tile([128, nt_, 128], BF16, tag="b16",
                              name=f"c_{key}_{t0}")
                nc.vector.tensor_copy(c[:], s[:])
                for i in range(nt_):
                    b0[("b16", key, t0 + i)] = (c, i)

            def b0_tpose(key, t):
                c, i = b0[("b16", key, t)]
                t_s = b0["T", key]
                ps = accp.tile([128, 128], BF16, tag="acc")
                nc.tensor.transpose(ps[:], c[:, i, :], ident[:])
                nc.vector.tensor_copy(t_s[:, ts(t, 128)], ps[:])

            def b0_load_v():
                va = big.tile([128, NT, D + 1], BF16, tag="va")
                nc.gpsimd.dma_start(
                    va[:, :, 0:D],
                    v_d[0].rearrange("(t p) d -> p t d", p=128),
                )
                nc.vector.memset(va[:, :, D : D + 1], 1.0)
                b0["va"] = va

            # ------------- steady batches: DMA-only setup --------------
            def make_setup_ops(b):
                # SWDGE casts f32->bf16 DRAM->DRAM, then the xbar
                # transpose-DMA lands Q^T/K^T directly in SBUF -- zero
                # PE/DVE work, long latency hidden by the issue lead.
                state = {}

                def load_tr(src_d, key):
                    scratch = dramp.tile(
                        [N, D], BF16, tag=key + "d", name=f"sc_{key}_{b}"
                    )
                    nc.gpsimd.dma_start(scratch[:], src_d[b][:])
                    t_s = big.tile([128, N], BF16, tag=key, name=f"ts_{key}_{b}")
                    nc.sync.dma_start(t_s[:], scratch[:], transpose=True)
                    state[key] = t_s

                def load_v():
                    va = big.tile([128, NT, D + 1], BF16, tag="va")
                    nc.gpsimd.dma_start(
                        va[:, :, 0:D],
                        v_d[b].rearrange("(t p) d -> p t d", p=128),
                    )
                    nc.vector.memset(va[:, :, D : D + 1], 1.0)
                    state["va"] = va

                def finish():
                    batch_tiles[b] = (state["qt"], state["kt"], state["va"])

                ops = [
                    lambda: load_tr(k_d, "kt"),
                    lambda: load_tr(q_d, "qt"),
                    load_v,
                ]
                return ops, finish

            # ---------------- MM2 chain queue --------------------------
            # Each job computes one 128-row output block of a finished
            # chunk; jobs are popped into later chunks' PE slack.
            chain_q = deque()

            def emit_chain(job):
                b, qc, qi, ptile, va, ot_all, meta = job
                o_ps = accp.tile([128, D + 1], FP32, tag="acc")
                for kt in range(NT):
                    nc.tensor.matmul(
                        o_ps[:],
                        ptile[:, kt, ts(qi, 128)],
                        va[:, kt, :],
                        start=(kt == 0),
                        stop=(kt == NT - 1),
                    )
                rec = smallp.tile([128, 1], FP32)
                nc.vector.reciprocal(rec[:], o_ps[:, D : D + 1])
                nc.vector.tensor_scalar_mul(ot_all[:, qi, :], o_ps[:, 0:D], rec[:])
                meta["done"] += 1
                if meta["done"] == NQC:
                    nc.sync.dma_start(
                        o_d[b, ts(qc, QCHUNK), :].rearrange(
                            "(c p) d -> p c d", p=128
                        ),
                        ot_all[:],
                    )

            def pop_chain(n=1):
                for _ in range(n):
                    if chain_q:
                        emit_chain(chain_q.popleft())

            def finish_chunk(b, qc, ptile, va, ot_all):
                meta = {"done": 0}
                for qi in range(QCHUNK // 128):
                    chain_q.append((b, qc, qi, ptile, va, ot_all, meta))

            # pending setup work: (ops, finish, deadline chunk index)
            pending = []

            def drip(ci, g, allow_pop):
                if pending:
                    ops, fin, deadline = pending[0]
                    n_slots = max(1, (deadline - ci) * len(GROUPS) - g)
                    take = max(1, -(-len(ops) // n_slots))
                    for op in ops[:take]:
                        op()
                    del ops[:take]
                    if not ops:
                        fin()
                        pending.pop(0)
                elif allow_pop and len(chain_q) > NQC:
                    pop_chain()

            def mm1_group(st, kt_s, qt_s, qc, k0, gs):
                for j in range(gs):
                    nc.tensor.matmul(
                        st[:, j, :],
                        kt_s[:, ts(k0 + j, 128)],
                        qt_s[:, ts(qc, QCHUNK)],
                        start=True,
                        stop=True,
                    )

            def exp_group(st, ptile, k0, gs):
                nc.scalar.activation(
                    ptile[:, k0 : k0 + gs, :],
                    st[:, 0:gs, :],
                    mybir.ActivationFunctionType.Exp,
                    scale=SCALE,
                )

            # ================= batch 0 ramp =============================
            # Small first-needed DMAs, identity while in flight, then
            # casts + PE transposes of kt0-5 / qt0-3 in consumption
            # order so the DVE FIFO never blocks the first exp.
            b0_load("kt", k_d, 0, 3)
            b0_load("qt", q_d, 0, 4)
            b0_load("kt", k_d, 3, 3)
            b0_load("qt", q_d, 4, 4)
            b0_load("kt", k_d, 6, 5)
            b0_load("kt", k_d, 11, 5)
            b0["T", "kt"] = big.tile([128, N], BF16, tag="kt", name="ts_kt_0")
            b0["T", "qt"] = big.tile([128, N], BF16, tag="qt", name="ts_qt_0")
            make_identity(nc, ident[:])
            b0_cast("kt", 0)
            for t in range(3):
                b0_tpose("kt", t)
            b0_cast("qt", 0)
            for t in range(4):
                b0_tpose("qt", t)
            b0_cast("kt", 3)
            for t in range(3, 6):
                b0_tpose("kt", t)
            b0_load_v()
            # Q tiles 8-15 via the DMA-only path (ready well before
            # q-chunks 2/3 need them); two transposes for finer arrival.
            qsc0 = dramp.tile([1024, D], BF16, tag="q0d", name="q0scr")
            nc.gpsimd.dma_start(qsc0[:], q_d[0, 1024:N, :])
            nc.sync.dma_start(
                b0["T", "qt"][:, 1024:1536], qsc0[0:512], transpose=True
            )
            nc.sync.dma_start(
                b0["T", "qt"][:, 1536:2048], qsc0[512:1024], transpose=True
            )
            batch_tiles[0] = (b0["T", "qt"], b0["T", "kt"], b0["va"])

            # remaining ramp work, dripped into phase-A slots with a
            # 2-group lead over the consuming MM1s
            ramp_ops = deque()
            ramp_ops.append(lambda: b0_cast("qt", 4))  # tiles 4-7
            for t in range(4, 8):
                ramp_ops.append(lambda t=t: b0_tpose("qt", t))
            ramp_ops.append(lambda: b0_cast("kt", 6))  # tiles 6-10
            for t in range(6, 11):
                ramp_ops.append(lambda t=t: b0_tpose("kt", t))
            ramp_ops.append(lambda: b0_cast("kt", 11))  # tiles 11-15
            for t in range(11, NT):
                ramp_ops.append(lambda t=t: b0_tpose("kt", t))

            # batch-1 setup DMAs issue early in phase A (their ~20us
            # latency then lands well before chunk C4 needs them).
            ops1, fin1 = make_setup_ops(1)

            qt0, kt0, va0 = batch_tiles[0]
            pt_a = [
                ptp.tile([128, NT, QCHUNK], BF16, tag="pt", name=f"pt{qc}")
                for qc in (0, 1)
            ]
            ot_a = [
                outp.tile([128, QCHUNK // 128, D], FP32, tag="ot", name=f"ot{qc}")
                for qc in (0, 1)
            ]
            # phase A: q-chunks 0 and 1 k-major, qc1 staggered one group
            # behind qc0 so dripped qt4-7 transposes land before qc1's
            # first MM1.  slot_plan = (group idx, qc, n drip ops).
            slot_plan = [
                (0, 0, 3), (1, 0, 2),
                (0, 1, 2), (1, 1, 2),
                (2, 0, 2), (2, 1, 2),
                (3, 0, 1), (3, 1, 1),
                (4, 0, 1), (4, 1, 1),
                (5, 0, 0), (5, 1, 0),
            ]
            for si, (gi, qc, n_drip) in enumerate(slot_plan):
                k0, gs = GROUPS[gi]
                st = stp.tile([128, 3, QCHUNK], FP32, tag="st")
                mm1_group(st, kt0, qt0, qc, k0, gs)
                exp_group(st, pt_a[qc], k0, gs)
                if si == 3:
                    for op in ops1:
                        op()
                    fin1()
                for _ in range(n_drip):
                    if ramp_ops:
                        ramp_ops.popleft()()
            assert not ramp_ops
            for qc in (0, 1):
                finish_chunk(0, qc, pt_a[qc], va0, ot_a[qc])

            # ================= steady chunks C2..C15 ====================
            chunks = [(0, 2), (0, 3)] + [
                (b, qc) for b in range(1, B_LOC) for qc in range(NQC)
            ]
            for ci, (b, qc) in enumerate(chunks, start=2):
                if qc == 1 and b + 1 < B_LOC:
                    ops, fin = make_setup_ops(b + 1)
                    pending.append((ops, fin, ci + 3))
                qt_s, kt_s, va = batch_tiles[b]
                ptile = ptp.tile([128, NT, QCHUNK], BF16, tag="pt")
                ot_all = outp.tile([128, QCHUNK // 128, D], FP32, tag="ot")
                for gi, (k0, gs) in enumerate(GROUPS):
                    st = stp.tile([128, 3, QCHUNK], FP32, tag="st")
                    mm1_group(st, kt_s, qt_s, qc, k0, gs)
                    exp_group(st, ptile, k0, gs)
                    if gi in (1, 2, 3, 4):
                        pop_chain()
                    else:
                        drip(ci, gi, allow_pop=(gi == 0))
                finish_chunk(b, qc, ptile, va, ot_all)

            # drain remaining MM2 chains
            pop_chain(len(chain_q))

    nc.compile()
    return nc


def _get_nc():
    if "nc" not in _CACHE:
        _CACHE["nc"] = build_nc()
    return _CACHE["nc"]


def run(q, k, v, **spmd_kwargs):
    """Run on all 8 cores; returns (full_output, BassKernelResults)."""
    nc = _get_nc()
    q = np.ascontiguousarray(q, dtype=np.float32)
    k = np.ascontiguousarray(k, dtype=np.float32)
    v = np.ascontiguousarray(v, dtype=np.float32)
    in_maps = [
        {
            "q": np.ascontiguousarray(q[i * B_LOC : (i + 1) * B_LOC]),
            "k": np.ascontiguousarray(k[i * B_LOC : (i + 1) * B_LOC]),
            "v": np.ascontiguousarray(v[i * B_LOC : (i + 1) * B_LOC]),
        }
        for i in range(N_CORES)
    ]
    res = run_bass_kernel_spmd(nc, in_maps, core_ids=list(range(N_CORES)), **spmd_kwargs)
    out = np.concatenate([r["out"] for r in res.results], axis=0)
    return out, res


def kernel(q, k, v):
    out, _ = run(q, k, v)
    return out


# revision 8
# speedup vs baseline: 1.0082x; 1.0082x over previous
"""Batch-parallel attention kernel for 8 TRN2 NeuronCores.

Problem: q,k,v [32, 2048, 128] f32 -> out = softmax(q@k^T/sqrt(128)) @ v.

Sharding: batch dim across 8 cores (4 batches/core), no cross-core comm.

Per-core algorithm (per batch, N=2048, D=128); ScalarE exp is the pacing
engine (16.8M exps at 1 elem/cycle/lane @1.2GHz = 109.2us floor):
  - Scores S^T[k, q] per 512-col q-chunk on PE (K^T tile stationary,
    Q^T chunk streaming, f32 PSUM), exp'd by ScalarE in SIX activations
    per chunk (k-tile groups 3,3,3,3,2,2): TRN2 PSUM matmul output must
    be f32, so a 3-k-tile group (1536 f32 = 3 banks) double-buffered
    (6 banks) plus 2 MM2-accumulator banks exactly fills the 8-bank
    PSUM.  Fewer/larger ACTIVATEs amortize the ~150ns/instr overhead.
  - V_aug [k, t, D+1] with a ones column makes the softmax denominator
    fall out of MM2 (column 128) -- no cross-partition reduction.
  - MM2: out[q,129] accumulated over 16 k-tiles with P^T[k,q] tiles
    stationary (FWL keeps the ~59ns/matmul cadence), then VectorE
    reciprocal+scale.  MM2 work is sliced into 4-matmul QUARTERS woven
    into every exp group (3/3/3/3/2/2 per group = 16/chunk = 4 chains)
    so the PE load per group stays under the exp duration -- chains as
    single 944ns bursts between MM1 groups stall ScalarE because the
    score PSUM is only double-buffered.
  - Batch 0 ramp is k-major: q-chunks 0 and 1 are produced interleaved
    (qc1 staggered one k-group behind qc0) so each PE K^T-transpose
    feeds TWO exps.  Only kt0-5 / qt0-3 transpose before the first exp,
    gated on just the first two staging DMAs; later K staging arrives
    in two more DMAs sized so each lands before its drip slot even
    under the concurrent SWDGE flood (Tile schedules all dependency-
    free DMA issues as early as it likes, so delivery order is shaped
    by SIZE and issue position, not emission order alone).  Q tiles
    8-15 via SWDGE f32->bf16 cast-DMA + xbar transpose-DMA (split in
    two for earlier partial arrival), as are batches 1-3 (batch 1 is
    emitted mid-phase-A so its flood follows the critical loads).
  - No max-subtraction: scores are ~N(0,1), exp is exact to ~2ulp on
    ScalarE and stays in range.

rel_l2 ~3e-3 vs f64 reference (bf16 operand rounding; f32 accumulation).
"""

import math
from collections import deque

import numpy as np

import concourse.bass as bass
import concourse.mybir as mybir
import concourse.tile as tile
from concourse import bacc
from concourse.bass import ts
from concourse.bass_utils import run_bass_kernel_spmd
from concourse.masks import make_identity

B, N, D = 32, 2048, 128
N_CORES = 8
B_LOC = B // N_CORES  # batches per core
NT = N // 128  # 16 k-tiles per batch
QCHUNK = 512
NQC = N // QCHUNK  # 4 q-chunks
SCALE = 1.0 / math.sqrt(D)
FP32 = mybir.dt.float32
BF16 = mybir.dt.bfloat16

# exp groups per chunk: (k-tile offset, n k-tiles). 3 k-tiles = 3 PSUM
# banks f32; double-buffered = 6 banks, + 2 MM2-acc banks = 8 total.
GROUPS = [(0, 3), (3, 3), (6, 3), (9, 3), (12, 2), (14, 2)]
QPG = [3, 3, 3, 3, 2, 2]  # MM2 quarters popped after each group

_CACHE = {}


def build_nc():
    nc = bacc.Bacc(None, target_bir_lowering=False)
    q_d = nc.dram_tensor("q", [B_LOC, N, D], FP32, kind="ExternalInput")
    k_d = nc.dram_tensor("k", [B_LOC, N, D], FP32, kind="ExternalInput")
    v_d = nc.dram_tensor("v", [B_LOC, N, D], FP32, kind="ExternalInput")
    o_d = nc.dram_tensor("out", [B_LOC, N, D], FP32, kind="ExternalOutput")

    with tile.TileContext(nc) as tc:
        with (
            tc.tile_pool(name="const", bufs=1) as constp,
            tc.tile_pool(name="dram", bufs=2, space="DRAM") as dramp,
            tc.tile_pool(name="stg", bufs=4) as stg,
            tc.tile_pool(name="b16", bufs=4) as b16p,
            tc.tile_pool(name="big", bufs=2) as big,
            tc.tile_pool(name="pt", bufs=3) as ptp,
            tc.tile_pool(name="outp", bufs=3) as outp,
            tc.tile_pool(name="small", bufs=8) as smallp,
            tc.tile_pool(name="st", bufs=2, space="PSUM") as stp,
            tc.tile_pool(name="acc", bufs=2, space="PSUM") as accp,
        ):
            ident = constp.tile([128, 128], BF16)

            batch_tiles = {}

            # ---------------- batch 0 ramp helpers (PE transposes) -----
            b0 = {}

            def b0_load(key, src_d, t0, nt_):
                s = stg.tile([128, nt_, 128], FP32, tag="stg",
                             name=f"s_{key}_{t0}")
                nc.sync.dma_start(
                    s[:],
                    src_d[0, bass.ds(t0 * 128, nt_ * 128), :].rearrange(
                        "(t p) d -> p t d", p=128
                    ),
                )
                b0[("f32", key, t0)] = s

            def b0_cast(key, t0, lo, n):
                # cast staging tiles [lo, lo+n) of the DMA that started
                # at tile t0 into their own bf16 tile
                s = b0[("f32", key, t0)]
                c = b16p.tile([128, n, 128], BF16, tag="b16",
                              name=f"c_{key}_{t0 + lo}")
                nc.vector.tensor_copy(c[:], s[:, lo : lo + n, :])
                for i in range(n):
                    b0[("b16", key, t0 + lo + i)] = (c, i)

            def b0_tpose(key, t):
                c, i = b0[("b16", key, t)]
                t_s = b0["T", key]
                ps = accp.tile([128, 128], BF16, tag="acc")
                nc.tensor.transpose(ps[:], c[:, i, :], ident[:])
                nc.vector.tensor_copy(t_s[:, ts(t, 128)], ps[:])

            def b0_load_v():
                va = big.tile([128, NT, D + 1], BF16, tag="va")
                nc.gpsimd.dma_start(
                    va[:, :, 0:D],
                    v_d[0].rearrange("(t p) d -> p t d", p=128),
                )
                nc.vector.memset(va[:, :, D : D + 1], 1.0)
                b0["va"] = va

            # ------------- steady batches: DMA-only setup --------------
            def make_setup_ops(b):
                # SWDGE casts f32->bf16 DRAM->DRAM, then the xbar
                # transpose-DMA lands Q^T/K^T directly in SBUF -- zero
                # PE/DVE work, long latency hidden by the issue lead.
                state = {}

                def load_tr(src_d, key):
                    scratch = dramp.tile(
                        [N, D], BF16, tag=key + "d", name=f"sc_{key}_{b}"
                    )
                    nc.gpsimd.dma_start(scratch[:], src_d[b][:])
                    t_s = big.tile([128, N], BF16, tag=key, name=f"ts_{key}_{b}")
                    nc.sync.dma_start(t_s[:], scratch[:], transpose=True)
                    state[key] = t_s

                def load_v():
                    va = big.tile([128, NT, D + 1], BF16, tag="va")
                    nc.gpsimd.dma_start(
                        va[:, :, 0:D],
                        v_d[b].rearrange("(t p) d -> p t d", p=128),
                    )
                    nc.vector.memset(va[:, :, D : D + 1], 1.0)
                    state["va"] = va

                def finish():
                    batch_tiles[b] = (state["qt"], state["kt"], state["va"])

                ops = [
                    lambda: load_tr(k_d, "kt"),
                    lambda: load_tr(q_d, "qt"),
                    load_v,
                ]
                return ops, finish

            # ---------------- MM2 quarter queue ------------------------
            # MM2 for one 128-row output block = 16 accumulating matmuls
            # + reciprocal/scale; sliced into 4-matmul quarters so pops
            # interleave finely with MM1 groups.
            quarter_q = deque()

            def emit_quarter(job):
                b, qc, qi, quarter, ptile, va, ot_all, meta = job
                if quarter == 0:
                    meta["o_ps"] = accp.tile(
                        [128, D + 1], FP32, tag="acc", name="o_ps"
                    )
                o_ps = meta["o_ps"]
                for kt in range(4 * quarter, 4 * quarter + 4):
                    nc.tensor.matmul(
                        o_ps[:],
                        ptile[:, kt, ts(qi, 128)],
                        va[:, kt, :],
                        start=(kt == 0),
                        stop=(kt == NT - 1),
                    )
                if quarter == 3:
                    rec = smallp.tile([128, 1], FP32)
                    nc.vector.reciprocal(rec[:], o_ps[:, D : D + 1])
                    nc.vector.tensor_scalar_mul(
                        ot_all[:, qi, :], o_ps[:, 0:D], rec[:]
                    )
                    meta["done"] += 1
                    if meta["done"] == NQC:
                        nc.sync.dma_start(
                            o_d[b, ts(qc, QCHUNK), :].rearrange(
                                "(c p) d -> p c d", p=128
                            ),
                            ot_all[:],
                        )

            def pop_quarters(n):
                for _ in range(n):
                    if quarter_q:
                        emit_quarter(quarter_q.popleft())

            def finish_chunk(b, qc, ptile, va, ot_all):
                meta = {"done": 0}
                for qi in range(QCHUNK // 128):
                    for quarter in range(4):
                        quarter_q.append(
                            (b, qc, qi, quarter, ptile, va, ot_all, meta)
                        )

            # pending setup work: (ops, finish, deadline chunk index)
            pending = []

            def drip(ci, g):
                if pending:
                    ops, fin, deadline = pending[0]
                    n_slots = max(1, (deadline - ci) * len(GROUPS) - g)
                    take = max(1, -(-len(ops) // n_slots))
                    for op in ops[:take]:
                        op()
                    del ops[:take]
                    if not ops:
                        fin()
                        pending.pop(0)

            def mm1_group(st, kt_s, qt_s, qc, k0, gs):
                for j in range(gs):
                    nc.tensor.matmul(
                        st[:, j, :],
                        kt_s[:, ts(k0 + j, 128)],
                        qt_s[:, ts(qc, QCHUNK)],
                        start=True,
                        stop=True,
                    )

            def exp_group(st, ptile, k0, gs):
                nc.scalar.activation(
                    ptile[:, k0 : k0 + gs, :],
                    st[:, 0:gs, :],
                    mybir.ActivationFunctionType.Exp,
                    scale=SCALE,
                )

            # ================= batch 0 ramp =============================
            # Four staging DMAs, critical-first; first exp gates on only
            # the first two.  K tiles 6-15 arrive in two pieces so each
            # lands before its drip slot even under the SWDGE flood.
            b0_load("kt", k_d, 0, 6)
            b0_load("qt", q_d, 0, 8)
            b0_load("kt", k_d, 6, 5)
            b0_load("kt", k_d, 11, 5)
            b0["T", "kt"] = big.tile([128, N], BF16, tag="kt", name="ts_kt_0")
            b0["T", "qt"] = big.tile([128, N], BF16, tag="qt", name="ts_qt_0")
            make_identity(nc, ident[:])
            b0_cast("kt", 0, 0, 6)
            for t in range(3):
                b0_tpose("kt", t)
            b0_cast("qt", 0, 0, 8)
            for t in range(4):
                b0_tpose("qt", t)
            for t in range(3, 6):
                b0_tpose("kt", t)
            # Q tiles 8-15 via the DMA-only path, cast split in two so
            # the first xbar transpose can start as soon as possible
            # (chunk C2 at ~31us needs qt8-11).
            qsc0 = dramp.tile([1024, D], BF16, tag="q0d", name="q0scr")
            nc.gpsimd.dma_start(qsc0[0:512], q_d[0, 1024:1536, :])
            nc.sync.dma_start(
                b0["T", "qt"][:, 1024:1536], qsc0[0:512], transpose=True
            )
            nc.gpsimd.dma_start(qsc0[512:1024], q_d[0, 1536:2048, :])
            nc.sync.dma_start(
                b0["T", "qt"][:, 1536:2048], qsc0[512:1024], transpose=True
            )
            b0_load_v()
            batch_tiles[0] = (b0["T", "qt"], b0["T", "kt"], b0["va"])

            # remaining ramp work, hand-placed into phase-A slots with a
            # 2-group lead over the consuming MM1s
            slot_ops = {
                0: [lambda: b0_tpose("qt", 4), lambda: b0_tpose("qt", 5),
                    lambda: b0_tpose("qt", 6)],
                1: [lambda: b0_tpose("qt", 7),
                    lambda: b0_cast("kt", 6, 0, 5)],
                2: [lambda: b0_tpose("kt", 6), lambda: b0_tpose("kt", 7)],
                3: [lambda: b0_tpose("kt", 8)],
                4: [lambda: b0_tpose("kt", 9), lambda: b0_tpose("kt", 10)],
                5: [lambda: b0_cast("kt", 11, 0, 5),
                    lambda: b0_tpose("kt", 11)],
                6: [lambda: b0_tpose("kt", 12)],
                7: [lambda: b0_tpose("kt", 13)],
                8: [lambda: b0_tpose("kt", 14)],
                9: [lambda: b0_tpose("kt", 15)],
            }

            # batch-1 setup DMAs: emitted mid-phase-A so their ~2MB of
            # SWDGE traffic queues behind the ramp-critical loads, while
            # still landing well before chunk C4 (~46us) needs them.
            ops1, fin1 = make_setup_ops(1)

            qt0, kt0, va0 = batch_tiles[0]
            pt_a = [
                ptp.tile([128, NT, QCHUNK], BF16, tag="pt", name=f"pt{qc}")
                for qc in (0, 1)
            ]
            ot_a = [
                outp.tile([128, QCHUNK // 128, D], FP32, tag="ot", name=f"ot{qc}")
                for qc in (0, 1)
            ]
            # phase A: q-chunks 0 and 1 k-major, qc1 staggered one group
            # behind qc0 so dripped qt4-7 transposes land before qc1's
            # first MM1.
            slot_plan = [
                (0, 0), (1, 0),
                (0, 1), (1, 1),
                (2, 0), (2, 1),
                (3, 0), (3, 1),
                (4, 0), (4, 1),
                (5, 0), (5, 1),
            ]
            for si, (gi, qc) in enumerate(slot_plan):
                k0, gs = GROUPS[gi]
                st = stp.tile([128, 3, QCHUNK], FP32, tag="st")
                mm1_group(st, kt0, qt0, qc, k0, gs)
                exp_group(st, pt_a[qc], k0, gs)
                if si == 5:
                    for op in ops1:
                        op()
                    fin1()
                for op in slot_ops.get(si, ()):
                    op()
            for qc in (0, 1):
                finish_chunk(0, qc, pt_a[qc], va0, ot_a[qc])

            # ================= steady chunks C2..C15 ====================
            chunks = [(0, 2), (0, 3)] + [
                (b, qc) for b in range(1, B_LOC) for qc in range(NQC)
            ]
            for ci, (b, qc) in enumerate(chunks, start=2):
                if qc == 1 and b + 1 < B_LOC:
                    ops, fin = make_setup_ops(b + 1)
                    pending.append((ops, fin, ci + 3))
                qt_s, kt_s, va = batch_tiles[b]
                ptile = ptp.tile([128, NT, QCHUNK], BF16, tag="pt")
                ot_all = outp.tile([128, QCHUNK // 128, D], FP32, tag="ot")
                for gi, (k0, gs) in enumerate(GROUPS):
                    st = stp.tile([128, 3, QCHUNK], FP32, tag="st")
                    mm1_group(st, kt_s, qt_s, qc, k0, gs)
                    exp_group(st, ptile, k0, gs)
                    pop_quarters(QPG[gi] + (1 if len(quarter_q) > 16 else 0))
                    if gi in (0, 5):
                        drip(ci, gi)
                finish_chunk(b, qc, ptile, va, ot_all)

            # drain remaining MM2 quarters
            pop_quarters(len(quarter_q))

    nc.compile()
    return nc


def _get_nc():
    if "nc" not in _CACHE:
        _CACHE["nc"] = build_nc()
    return _CACHE["nc"]


def run(q, k, v, **spmd_kwargs):
    """Run on all 8 cores; returns (full_output, BassKernelResults)."""
    nc = _get_nc()
    q = np.ascontiguousarray(q, dtype=np.float32)
    k = np.ascontiguousarray(k, dtype=np.float32)
    v = np.ascontiguousarray(v, dtype=np.float32)
    in_maps = [
        {
            "q": np.ascontiguousarray(q[i * B_LOC : (i + 1) * B_LOC]),
            "k": np.ascontiguousarray(k[i * B_LOC : (i + 1) * B_LOC]),
            "v": np.ascontiguousarray(v[i * B_LOC : (i + 1) * B_LOC]),
        }
        for i in range(N_CORES)
    ]
    res = run_bass_kernel_spmd(nc, in_maps, core_ids=list(range(N_CORES)), **spmd_kwargs)
    out = np.concatenate([r["out"] for r in res.results], axis=0)
    return out, res


def kernel(q, k, v):
    out, _ = run(q, k, v)
    return out
